# revision 17
# baseline (speedup 1.0000x reference)
"""Bass/Tile Trainium2 kernel for nn_Attention_7284264534326.

Single-head attention, B=8, S=2048, D=1024:
    q = (x1 @ wq) * D**-0.5 ; k = x2 @ wk ; v = x2 @ wv
    a = softmax(q @ k^T + mask * -1e9, axis=-1)
    out = relu(a @ v) @ wo

Sharding: data-parallel over batch; one batch element per NeuronCore (8 cores).

Structural optimizations vs a direct implementation:
  - maskSeq is per-KEY and constant across queries, and exp(x - 1e9) == 0
    exactly in f32.  Masked keys contribute nothing, so the host compacts x2
    to its unmasked rows (padded to K_CAP, a multiple of 128), shrinking the
    k/v projections, score matmul, and a@v matmul from S=2048 keys to K_CAP
    (~1152 for ~50% masking).  Pad keys get an additive -1e9 bias so their
    exp is exactly 0.
  - Associativity fold: scores = (x1 wq s)(x2c wk)^T = x1 G x2c^T with
    G = s * wq wk^T computed on the HOST.  The device computes
    T1 = x2c @ G^T (over the ~1152 compacted keys only) and then
    scores^T = T1^T-contraction against x1^T directly — the entire
    q-projection GEMM disappears from the device.
  - x1 and compacted-x2 are transposed to [D, *] layout on the host, so the
    kernel needs no PE transposes: every matmul operand arrives in
    [contraction-on-partitions, free] layout.
  - Per-core dataflow (all matmul operands bf16, PSUM accumulation f32):
      T1T[d',k] = matmul(lhsT=G[d,d'], rhs=x2cT[d,k]);  V[k,e] likewise
      from wv.  Per 512-query tile:
      scores^T[k,q] = matmul(lhsT=T1T[:,d',k-blk], rhs=x1T[d',q]); exp fused
      into ACT evacuation with the additive mask as a per-partition bias.
      softmax denominator: DVE sums the exp chunk tiles in f32, then one
      tiny f32 matmul per 128-query block against a ones-vector reduces over
      partitions (keeps the heavy reduction off the PE).
      yU^T[e,q] = matmul(lhsT=V, rhs=exp^T); relu on ACT evacuation
      (normalization deferred: relu(y/d) == relu(y)/d for d>0).
      out[q,f] = matmul(lhsT=relu^T, rhs=wo) scaled by 1/denom on evacuation.
"""

import numpy as np
from contextlib import ExitStack

B, S, D = 8, 2048, 1024
P = 128
DC = D // P       # 8 chunks of the depth/contraction dim
EC = D // P       # 8 chunks of the embedding dim
Q_TILE = 512      # queries per tile (max moving free dim)
NQT = S // Q_TILE # 4
N_CORES = 8
QSCALE = float(D) ** -0.5  # folded into G on the host

_nc_cache = {}


def _build(k_cap):
    import concourse.tile as tile
    from concourse import bacc, mybir

    f32 = mybir.dt.float32
    bf16 = mybir.dt.bfloat16
    AF = mybir.ActivationFunctionType
    KC = k_cap // P

    nc = bacc.Bacc("TRN2", target_bir_lowering=False, debug=False,
                   enable_asserts=False, num_devices=N_CORES)

    x1t = nc.dram_tensor("x1t", [D, S], bf16, kind="ExternalInput").ap()
    x2t = nc.dram_tensor("x2t", [D, k_cap], bf16, kind="ExternalInput").ap()
    g = nc.dram_tensor("g", [D, D], bf16, kind="ExternalInput").ap()
    wv = nc.dram_tensor("wv", [D, D], bf16, kind="ExternalInput").ap()
    wo = nc.dram_tensor("wo", [D, D], bf16, kind="ExternalInput").ap()
    mb = nc.dram_tensor("mb", [P, KC], f32, kind="ExternalInput").ap()
    out = nc.dram_tensor("out", [S, D], f32, kind="ExternalOutput").ap()

    with tile.TileContext(nc) as tc, ExitStack() as ctx:
        persist = ctx.enter_context(tc.tile_pool(name="persist", bufs=1))

        t1 = persist.tile([P, DC, k_cap], bf16, name="t1")   # [d', k] by d'-chunk
        V = persist.tile([P, KC, D], bf16, name="V")         # [k, e] by k-chunk
        wo_sb = persist.tile([P, DC, D], bf16, name="wo_sb")
        maskbias = persist.tile([P, KC], f32, name="maskbias")
        ones_f = persist.tile([P, 1], f32, name="ones_f")

        x1pool = ctx.enter_context(tc.tile_pool(name="x1pool", bufs=2))
        ppsum = ctx.enter_context(tc.tile_pool(name="ppsum", bufs=2, space="PSUM"))

        nc.vector.memset(ones_f, 1.0)

        x1tiles = {}

        def load_x1(qt, eng=None):
            t = x1pool.tile([P, DC, Q_TILE], bf16, name="x1s", tag="x1s")
            (eng or nc.sync).dma_start(
                t, x1t[:, qt * Q_TILE:(qt + 1) * Q_TILE].rearrange(
                    "(c p) s -> p c s", p=P))
            x1tiles[qt] = t

        # ============ phase 1: T1 and V (x2-path weights scoped) ============
        with ExitStack() as p1:
            kvpool = p1.enter_context(tc.tile_pool(name="kvpool", bufs=1))
            x2sb = kvpool.tile([P, DC, k_cap], bf16, name="x2sb")
            g_sb = kvpool.tile([P, DC, D], bf16, name="g_sb")
            wv_sb = kvpool.tile([P, DC, D], bf16, name="wv_sb")

            # T1 key-slices: near-equal widths <= 512.  Keeping every slice
            # >= ~256 wide matters: a matmul's LDWEIGHTS (128 cy) only hides
            # under a stream of >= its length, so short slices run at half
            # rate.  (1152 -> 3x384.)
            nsl = -(-k_cap // 512)
            base, rem = divmod(k_cap, nsl)
            kslices = []
            k0 = 0
            for i in range(nsl):
                w = base + (1 if i < rem else 0)
                kslices.append((k0, w))
                k0 += w

            # Per-queue DMA sustains only ~140-160 GB/s (gpsimd's software
            # DGE is the slowest), so schedule the loads on explicit queues
            # in priority order.  The first T1 chains (ec 0..3, slice0) need
            # only G cols 0:512 (split in eighths over the two fast HW
            # queues) + the small x2 slice0 (gpsimd).
            SY, SC, GP = nc.sync, nc.scalar, nc.gpsimd
            for dc in range(DC):
                (SY if dc % 2 == 0 else SC).dma_start(
                    g_sb[:, dc, 0:512], g[dc * P:(dc + 1) * P, 0:512])
            for k0, w in kslices:
                for dh in range(4):
                    (SY, SC, GP, GP)[dh].dma_start(
                        x2sb[:, 2 * dh:2 * dh + 2, k0:k0 + w],
                        x2t[2 * dh * P:(2 * dh + 2) * P, k0:k0 + w]
                        .rearrange("(c p) k -> p c k", p=P))
            for dh in range(4):
                (SY if dh % 2 == 0 else SC).dma_start(
                    g_sb[:, 2 * dh:2 * dh + 2, 512:1024],
                    g[2 * dh * P:(2 * dh + 2) * P, 512:1024]
                    .rearrange("(c p) e -> p c e", p=P))
            for dh, eng in enumerate((SY, SC, GP, GP)):
                eng.dma_start(
                    wv_sb[:, 2 * dh:2 * dh + 2, :],
                    wv[2 * dh * P:(2 * dh + 2) * P, :]
                    .rearrange("(c p) e -> p c e", p=P))
            load_x1(0)  # sync: overlaps phase 1 compute
            GP.dma_start(maskbias, mb)
            for dh, eng in enumerate((SY, SC)):
                eng.dma_start(
                    wo_sb[:, 4 * dh:4 * dh + 4, :],
                    wo[4 * dh * P:(4 * dh + 4) * P, :]
                    .rearrange("(c p) e -> p c e", p=P))

            # eh0-half chains first: they only need G columns 0:512, so the
            # PE can run while G's other half and wv still stream in.
            for eh in range(2):
                for k0, w in kslices:
                    for ec in range(4 * eh, 4 * eh + 4):
                        pq = ppsum.tile([P, 512], f32, name="pq", tag="pp")
                        for dc in range(DC):
                            nc.tensor.matmul(
                                pq[:, :w],
                                lhsT=g_sb[:, dc, ec * P:(ec + 1) * P],
                                rhs=x2sb[:, dc, k0:k0 + w],
                                start=(dc == 0), stop=(dc == DC - 1))
                        nc.scalar.activation(
                            out=t1[:, ec, k0:k0 + w], in_=pq[:, :w],
                            func=AF.Copy)

            for kc in range(KC):
                for eh in range(2):
                    pv = ppsum.tile([P, 512], f32, name="pq", tag="pp")
                    for dc in range(DC):
                        nc.tensor.matmul(
                            pv, lhsT=x2sb[:, dc, kc * P:(kc + 1) * P],
                            rhs=wv_sb[:, dc, eh * 512:(eh + 1) * 512],
                            start=(dc == 0), stop=(dc == DC - 1))
                    # DVE evacuation keeps the ACT engine free for T1/exp
                    nc.vector.tensor_copy(
                        out=V[:, kc, eh * 512:(eh + 1) * 512], in_=pv)

        # ============ phase 2: per-512-query-tile attention ============
        epool = ctx.enter_context(tc.tile_pool(name="epool", bufs=2))
        zpool = ctx.enter_context(tc.tile_pool(name="zpool", bufs=2))
        opool = ctx.enter_context(tc.tile_pool(name="opool", bufs=3))
        dpool = ctx.enter_context(tc.tile_pool(name="dpool", bufs=2))
        rpool = ctx.enter_context(tc.tile_pool(name="rpool", bufs=2))
        spsum = ctx.enter_context(tc.tile_pool(name="spsum", bufs=2, space="PSUM"))
        ypsum = ctx.enter_context(tc.tile_pool(name="ypsum", bufs=2, space="PSUM"))
        opsum = ctx.enter_context(tc.tile_pool(name="opsum", bufs=2, space="PSUM"))

        for qt in range(NQT):
            q0 = qt * Q_TILE
            if qt + 1 < NQT:
                load_x1(qt + 1)
            x1s = x1tiles.pop(qt)

            # scores^T + fused exp(mask-biased): contraction over d' with
            # T1 stationary and the raw x1^T slice moving
            expt = epool.tile([P, KC, Q_TILE], bf16, name="expt", tag="expt")
            for kc in range(KC):
                ps = spsum.tile([P, Q_TILE], f32, name="ps", tag="ps")
                for dc in range(DC):
                    nc.tensor.matmul(
                        ps, lhsT=t1[:, dc, kc * P:(kc + 1) * P],
                        rhs=x1s[:, dc, :],
                        start=(dc == 0), stop=(dc == DC - 1))
                nc.scalar.activation(
                    out=expt[:, kc, :], in_=ps, func=AF.Exp,
                    bias=maskbias[:, kc:kc + 1], scale=1.0)

            # partial softmax denominators on DVE (f32), [k-part, q]
            dsum = dpool.tile([P, Q_TILE], f32, name="dsum", tag="dsum")
            if KC == 1:
                nc.vector.tensor_copy(out=dsum, in_=expt[:, 0, :])
            else:
                nc.vector.tensor_add(dsum, expt[:, 0, :], expt[:, 1, :])
                for kc in range(2, KC):
                    nc.vector.tensor_add(dsum, dsum, expt[:, kc, :])

            # yU^T = V^T @ exp^T, relu on evacuation
            zt = zpool.tile([P, EC, Q_TILE], bf16, name="zt", tag="zt")
            for ec in range(EC):
                py = ypsum.tile([P, Q_TILE], f32, name="py", tag="py")
                for kc in range(KC):
                    nc.tensor.matmul(
                        py, lhsT=V[:, kc, ec * P:(ec + 1) * P],
                        rhs=expt[:, kc, :],
                        start=(kc == 0), stop=(kc == KC - 1))
                nc.scalar.activation(out=zt[:, ec, :], in_=py, func=AF.Relu)

            # finish the denominators: reduce dsum over partitions per
            # 128-query block (f32 matmul against ones; borrows an opsum slot)
            pd = opsum.tile([P, 512], f32, name="po", tag="po")
            for qs in range(Q_TILE // P):
                nc.tensor.matmul(
                    pd[:, qs:qs + 1], lhsT=dsum[:, qs * P:(qs + 1) * P],
                    rhs=ones_f, start=True, stop=True)
            recip = rpool.tile([P, Q_TILE // P], f32, name="recip", tag="recip")
            nc.vector.reciprocal(recip, pd[:, 0:Q_TILE // P])

            # output projection, normalized on evacuation
            for qs in range(Q_TILE // P):
                osb = opool.tile([P, D], f32, name="osb", tag="osb")
                for fh in range(2):
                    po = opsum.tile([P, 512], f32, name="po", tag="po")
                    for ec in range(EC):
                        nc.tensor.matmul(
                            po, lhsT=zt[:, ec, qs * P:(qs + 1) * P],
                            rhs=wo_sb[:, ec, fh * 512:(fh + 1) * 512],
                            start=(ec == 0), stop=(ec == EC - 1))
                    nc.scalar.activation(
                        out=osb[:, fh * 512:(fh + 1) * 512], in_=po,
                        func=AF.Copy, scale=recip[:, qs:qs + 1])
                    # per-half store: the DMA overlaps the other half's evac
                    nc.sync.dma_start(
                        out[q0 + qs * P: q0 + (qs + 1) * P,
                            fh * 512:(fh + 1) * 512],
                        osb[:, fh * 512:(fh + 1) * 512])

    nc.compile()
    return nc


def _prepare(x1, x2, maskSeq, wq, wk, wv, wo):
    """Host-side prep: compact keys, fold wq@wk^T, transpose activations."""
    import ml_dtypes
    bf = ml_dtypes.bfloat16

    x1 = np.asarray(x1, dtype=np.float32)
    x2 = np.asarray(x2, dtype=np.float32)
    msk = np.asarray(maskSeq, dtype=np.int32).reshape(B, S)
    counts = [int(np.count_nonzero(msk[c] == 0)) for c in range(B)]
    k_cap = max(P, -(-max(counts) // P) * P)
    KC = k_cap // P

    if k_cap not in _nc_cache:
        _nc_cache[k_cap] = _build(k_cap)
    nc = _nc_cache[k_cap]

    # device computes T1 = x2c @ G^T from g's rows as the contraction dim,
    # so send G^T = s * wk @ wq^T
    g_f = QSCALE * (np.asarray(wk, dtype=np.float32) @
                    np.asarray(wq, dtype=np.float32).T)
    g_b = np.ascontiguousarray(g_f.astype(bf))
    wv_b = np.ascontiguousarray(np.asarray(wv, dtype=np.float32).astype(bf))
    wo_b = np.ascontiguousarray(np.asarray(wo, dtype=np.float32).astype(bf))

    kidx = np.arange(k_cap).reshape(KC, P).T  # kidx[p, kc] = kc*128 + p
    in_maps = []
    for c in range(B):
        idx = np.flatnonzero(msk[c] == 0)
        x2c = np.zeros((k_cap, D), dtype=np.float32)
        x2c[:len(idx)] = x2[c][idx]
        in_maps.append({
            "x1t": np.ascontiguousarray(x1[c].T.astype(bf)),
            "x2t": np.ascontiguousarray(x2c.T.astype(bf)),
            "g": g_b, "wv": wv_b, "wo": wo_b,
            "mb": np.where(kidx < len(idx), np.float32(0.0),
                           np.float32(-1.0e9)),
        })
    return nc, in_maps


def kernel(x1, x2, maskSeq, wq, wk, wv, wo, **_unused):
    from concourse.bass_utils import run_bass_kernel_spmd

    nc, in_maps = _prepare(x1, x2, maskSeq, wq, wk, wv, wo)
    res = run_bass_kernel_spmd(nc, in_maps, core_ids=list(range(N_CORES)))
    return np.stack([res.results[c]["out"] for c in range(N_CORES)], axis=0)


# revision 18
# speedup vs baseline: 1.1905x; 1.1905x over previous
"""Bass/Tile Trainium2 kernel for nn_Attention_7284264534326.

Single-head attention, B=8, S=2048, D=1024:
    q = (x1 @ wq) * D**-0.5 ; k = x2 @ wk ; v = x2 @ wv
    a = softmax(q @ k^T + mask * -1e9, axis=-1)
    out = relu(a @ v) @ wo

Sharding: data-parallel over batch; one batch element per NeuronCore (8 cores).

Structural optimizations vs a direct implementation:
  - maskSeq is per-KEY and constant across queries, and exp(x - 1e9) == 0
    exactly in f32.  Masked keys contribute nothing, so the host compacts x2
    to its unmasked rows (padded to K_CAP, a multiple of 128), shrinking the
    k/v projections, score matmul, and a@v matmul from S=2048 keys to K_CAP
    (~1152 for ~50% masking).  Pad keys get an additive -1e9 bias so their
    exp is exactly 0.
  - Associativity fold: scores = (x1 wq s)(x2c wk)^T = x1 G x2c^T with
    G = s * wq wk^T computed on the HOST.  The device computes
    T1 = x2c @ G^T (over the ~1152 compacted keys only) and then
    scores^T = T1^T-contraction against x1^T directly — the entire
    q-projection GEMM disappears from the device.
  - x1 and compacted-x2 are transposed to [D, *] layout on the host, so the
    kernel needs no PE transposes: every matmul operand arrives in
    [contraction-on-partitions, free] layout.
  - Per-core dataflow (all matmul operands bf16, PSUM accumulation f32):
      T1T[d',k] = matmul(lhsT=G[d,d'], rhs=x2cT[d,k]);  V[k,e] likewise
      from wv.  Per 512-query tile:
      scores^T[k,q] = matmul(lhsT=T1T[:,d',k-blk], rhs=x1T[d',q]); exp fused
      into ACT evacuation with the additive mask as a per-partition bias.
      softmax denominator: DVE sums the exp chunk tiles in f32, then one
      tiny f32 matmul per 128-query block against a ones-vector reduces over
      partitions (keeps the heavy reduction off the PE).
      yU^T[e,q] = matmul(lhsT=V, rhs=exp^T); relu on ACT evacuation
      (normalization deferred: relu(y/d) == relu(y)/d for d>0).
      out[q,f] = matmul(lhsT=relu^T, rhs=wo) scaled by 1/denom on evacuation.
"""

import numpy as np
from contextlib import ExitStack

B, S, D = 8, 2048, 1024
P = 128
DC = D // P       # 8 chunks of the depth/contraction dim
EC = D // P       # 8 chunks of the embedding dim
Q_TILE = 512      # queries per tile (max moving free dim)
NQT = S // Q_TILE # 4
N_CORES = 8
QSCALE = float(D) ** -0.5  # folded into G on the host

_nc_cache = {}


def _build(k_cap):
    import concourse.tile as tile
    from concourse import bacc, mybir

    f32 = mybir.dt.float32
    bf16 = mybir.dt.bfloat16
    AF = mybir.ActivationFunctionType
    KC = k_cap // P

    nc = bacc.Bacc("TRN2", target_bir_lowering=False, debug=False,
                   enable_asserts=False, num_devices=N_CORES)

    x1t = nc.dram_tensor("x1t", [D, S], bf16, kind="ExternalInput").ap()
    x2t = nc.dram_tensor("x2t", [D, k_cap], bf16, kind="ExternalInput").ap()
    g = nc.dram_tensor("g", [D, D], bf16, kind="ExternalInput").ap()
    wv = nc.dram_tensor("wv", [D, D], bf16, kind="ExternalInput").ap()
    wo = nc.dram_tensor("wo", [D, D], bf16, kind="ExternalInput").ap()
    mb = nc.dram_tensor("mb", [P, KC], f32, kind="ExternalInput").ap()
    out = nc.dram_tensor("out", [S, D], f32, kind="ExternalOutput").ap()

    with tile.TileContext(nc) as tc, ExitStack() as ctx:
        persist = ctx.enter_context(tc.tile_pool(name="persist", bufs=1))

        t1 = persist.tile([P, DC, k_cap], bf16, name="t1")   # [d', k] by d'-chunk
        V = persist.tile([P, KC, D], bf16, name="V")         # [k, e] by k-chunk
        wo_sb = persist.tile([P, DC, D], bf16, name="wo_sb")
        maskbias = persist.tile([P, KC], f32, name="maskbias")
        ones_f = persist.tile([P, 1], f32, name="ones_f")

        x1pool = ctx.enter_context(tc.tile_pool(name="x1pool", bufs=2))
        ppsum = ctx.enter_context(tc.tile_pool(name="ppsum", bufs=2, space="PSUM"))

        nc.vector.memset(ones_f, 1.0)

        x1tiles = {}

        def load_x1(qt, eng=None):
            t = x1pool.tile([P, DC, Q_TILE], bf16, name="x1s", tag="x1s")
            (eng or nc.sync).dma_start(
                t, x1t[:, qt * Q_TILE:(qt + 1) * Q_TILE].rearrange(
                    "(c p) s -> p c s", p=P))
            x1tiles[qt] = t

        # ============ phase 1: T1 and V (x2-path weights scoped) ============
        with ExitStack() as p1:
            kvpool = p1.enter_context(tc.tile_pool(name="kvpool", bufs=1))
            x2sb = kvpool.tile([P, DC, k_cap], bf16, name="x2sb")
            g_sb = kvpool.tile([P, DC, D], bf16, name="g_sb")
            wv_sb = kvpool.tile([P, DC, D], bf16, name="wv_sb")

            # T1 key-slices: near-equal widths <= 512.  Keeping every slice
            # >= ~256 wide matters: a matmul's LDWEIGHTS (128 cy) only hides
            # under a stream of >= its length, so short slices run at half
            # rate.  (1152 -> 3x384.)
            nsl = -(-k_cap // 512)
            base, rem = divmod(k_cap, nsl)
            kslices = []
            k0 = 0
            for i in range(nsl):
                w = base + (1 if i < rem else 0)
                kslices.append((k0, w))
                k0 += w

            # Per-queue DMA sustains only ~140-160 GB/s (gpsimd's software
            # DGE is the slowest), so schedule the loads on explicit queues
            # in priority order.  The first T1 chains (ec 0..3, slice0) need
            # only G cols 0:512 (split in eighths over the two fast HW
            # queues) + the small x2 slice0 (gpsimd).
            SY, SC, GP = nc.sync, nc.scalar, nc.gpsimd

            def x2_piece(k0, w, dh, eng):
                eng.dma_start(
                    x2sb[:, 2 * dh:2 * dh + 2, k0:k0 + w],
                    x2t[2 * dh * P:(2 * dh + 2) * P, k0:k0 + w]
                    .rearrange("(c p) k -> p c k", p=P))

            # interleave G cols 0:512 with x2 slice0 so both land together
            k0c, wc = kslices[0]
            x2_piece(k0c, wc, 2, GP)
            x2_piece(k0c, wc, 3, GP)
            for dc in range(DC):
                (SY if dc % 2 == 0 else SC).dma_start(
                    g_sb[:, dc, 0:512], g[dc * P:(dc + 1) * P, 0:512])
                if dc == 0:
                    x2_piece(k0c, wc, 0, SY)
                elif dc == 1:
                    x2_piece(k0c, wc, 1, SC)
            for k0, w in kslices[1:]:
                for dh in range(4):
                    x2_piece(k0, w, dh, (SY, SC, GP, GP)[dh])
            for dh in range(4):
                (SY if dh % 2 == 0 else SC).dma_start(
                    g_sb[:, 2 * dh:2 * dh + 2, 512:1024],
                    g[2 * dh * P:(2 * dh + 2) * P, 512:1024]
                    .rearrange("(c p) e -> p c e", p=P))
            for dh, eng in enumerate((SY, SC, GP, GP)):
                eng.dma_start(
                    wv_sb[:, 2 * dh:2 * dh + 2, :],
                    wv[2 * dh * P:(2 * dh + 2) * P, :]
                    .rearrange("(c p) e -> p c e", p=P))
            load_x1(0)  # sync: overlaps phase 1 compute
            GP.dma_start(maskbias, mb)
            for dh, eng in enumerate((SY, SC)):
                eng.dma_start(
                    wo_sb[:, 4 * dh:4 * dh + 4, :],
                    wo[4 * dh * P:(4 * dh + 4) * P, :]
                    .rearrange("(c p) e -> p c e", p=P))

            # eh0-half chains first: they only need G columns 0:512, so the
            # PE can run while G's other half and wv still stream in.
            for eh in range(2):
                for k0, w in kslices:
                    for ec in range(4 * eh, 4 * eh + 4):
                        pq = ppsum.tile([P, 512], f32, name="pq", tag="pp")
                        for dc in range(DC):
                            nc.tensor.matmul(
                                pq[:, :w],
                                lhsT=g_sb[:, dc, ec * P:(ec + 1) * P],
                                rhs=x2sb[:, dc, k0:k0 + w],
                                start=(dc == 0), stop=(dc == DC - 1))
                        nc.scalar.activation(
                            out=t1[:, ec, k0:k0 + w], in_=pq[:, :w],
                            func=AF.Copy)

            for kc in range(KC):
                for eh in range(2):
                    pv = ppsum.tile([P, 512], f32, name="pq", tag="pp")
                    for dc in range(DC):
                        nc.tensor.matmul(
                            pv, lhsT=x2sb[:, dc, kc * P:(kc + 1) * P],
                            rhs=wv_sb[:, dc, eh * 512:(eh + 1) * 512],
                            start=(dc == 0), stop=(dc == DC - 1))
                    # DVE evacuation keeps the ACT engine free for T1/exp
                    nc.vector.tensor_copy(
                        out=V[:, kc, eh * 512:(eh + 1) * 512], in_=pv)

        # ============ phase 2: per-512-query-tile attention ============
        epool = ctx.enter_context(tc.tile_pool(name="epool", bufs=2))
        zpool = ctx.enter_context(tc.tile_pool(name="zpool", bufs=2))
        opool = ctx.enter_context(tc.tile_pool(name="opool", bufs=3))
        dpool = ctx.enter_context(tc.tile_pool(name="dpool", bufs=2))
        rpool = ctx.enter_context(tc.tile_pool(name="rpool", bufs=2))
        spsum = ctx.enter_context(tc.tile_pool(name="spsum", bufs=2, space="PSUM"))
        ypsum = ctx.enter_context(tc.tile_pool(name="ypsum", bufs=2, space="PSUM"))
        opsum = ctx.enter_context(tc.tile_pool(name="opsum", bufs=2, space="PSUM"))

        for qt in range(NQT):
            q0 = qt * Q_TILE
            if qt + 1 < NQT:
                load_x1(qt + 1)
            x1s = x1tiles.pop(qt)

            # scores^T + fused exp(mask-biased): contraction over d' with
            # T1 stationary and the raw x1^T slice moving
            expt = epool.tile([P, KC, Q_TILE], bf16, name="expt", tag="expt")
            for kc in range(KC):
                ps = spsum.tile([P, Q_TILE], f32, name="ps", tag="ps")
                for dc in range(DC):
                    nc.tensor.matmul(
                        ps, lhsT=t1[:, dc, kc * P:(kc + 1) * P],
                        rhs=x1s[:, dc, :],
                        start=(dc == 0), stop=(dc == DC - 1))
                nc.scalar.activation(
                    out=expt[:, kc, :], in_=ps, func=AF.Exp,
                    bias=maskbias[:, kc:kc + 1], scale=1.0)

            # partial softmax denominators on DVE (f32), [k-part, q]
            dsum = dpool.tile([P, Q_TILE], f32, name="dsum", tag="dsum")
            if KC == 1:
                nc.vector.tensor_copy(out=dsum, in_=expt[:, 0, :])
            else:
                nc.vector.tensor_add(dsum, expt[:, 0, :], expt[:, 1, :])
                for kc in range(2, KC):
                    nc.vector.tensor_add(dsum, dsum, expt[:, kc, :])

            # yU^T = V^T @ exp^T, relu on evacuation
            zt = zpool.tile([P, EC, Q_TILE], bf16, name="zt", tag="zt")
            for ec in range(EC):
                py = ypsum.tile([P, Q_TILE], f32, name="py", tag="py")
                for kc in range(KC):
                    nc.tensor.matmul(
                        py, lhsT=V[:, kc, ec * P:(ec + 1) * P],
                        rhs=expt[:, kc, :],
                        start=(kc == 0), stop=(kc == KC - 1))
                nc.scalar.activation(out=zt[:, ec, :], in_=py, func=AF.Relu)

            # finish the denominators: reduce dsum over partitions per
            # 128-query block (f32 matmul against ones; borrows an opsum slot)
            pd = opsum.tile([P, 512], f32, name="po", tag="po")
            for qs in range(Q_TILE // P):
                nc.tensor.matmul(
                    pd[:, qs:qs + 1], lhsT=dsum[:, qs * P:(qs + 1) * P],
                    rhs=ones_f, start=True, stop=True)
            recip = rpool.tile([P, Q_TILE // P], f32, name="recip", tag="recip")
            nc.vector.reciprocal(recip, pd[:, 0:Q_TILE // P])

            # output projection, normalized on evacuation
            for qs in range(Q_TILE // P):
                osb = opool.tile([P, D], f32, name="osb", tag="osb")
                for fh in range(2):
                    po = opsum.tile([P, 512], f32, name="po", tag="po")
                    for ec in range(EC):
                        nc.tensor.matmul(
                            po, lhsT=zt[:, ec, qs * P:(qs + 1) * P],
                            rhs=wo_sb[:, ec, fh * 512:(fh + 1) * 512],
                            start=(ec == 0), stop=(ec == EC - 1))
                    nc.scalar.activation(
                        out=osb[:, fh * 512:(fh + 1) * 512], in_=po,
                        func=AF.Copy, scale=recip[:, qs:qs + 1])
                    # per-half store: the DMA overlaps the other half's evac
                    nc.sync.dma_start(
                        out[q0 + qs * P: q0 + (qs + 1) * P,
                            fh * 512:(fh + 1) * 512],
                        osb[:, fh * 512:(fh + 1) * 512])

    nc.compile()
    return nc


def _prepare(x1, x2, maskSeq, wq, wk, wv, wo):
    """Host-side prep: compact keys, fold wq@wk^T, transpose activations."""
    import ml_dtypes
    bf = ml_dtypes.bfloat16

    x1 = np.asarray(x1, dtype=np.float32)
    x2 = np.asarray(x2, dtype=np.float32)
    msk = np.asarray(maskSeq, dtype=np.int32).reshape(B, S)
    counts = [int(np.count_nonzero(msk[c] == 0)) for c in range(B)]
    k_cap = max(P, -(-max(counts) // P) * P)
    KC = k_cap // P

    if k_cap not in _nc_cache:
        _nc_cache[k_cap] = _build(k_cap)
    nc = _nc_cache[k_cap]

    # device computes T1 = x2c @ G^T from g's rows as the contraction dim,
    # so send G^T = s * wk @ wq^T
    g_f = QSCALE * (np.asarray(wk, dtype=np.float32) @
                    np.asarray(wq, dtype=np.float32).T)
    g_b = np.ascontiguousarray(g_f.astype(bf))
    wv_b = np.ascontiguousarray(np.asarray(wv, dtype=np.float32).astype(bf))
    wo_b = np.ascontiguousarray(np.asarray(wo, dtype=np.float32).astype(bf))

    kidx = np.arange(k_cap).reshape(KC, P).T  # kidx[p, kc] = kc*128 + p
    in_maps = []
    for c in range(B):
        idx = np.flatnonzero(msk[c] == 0)
        x2c = np.zeros((k_cap, D), dtype=np.float32)
        x2c[:len(idx)] = x2[c][idx]
        in_maps.append({
            "x1t": np.ascontiguousarray(x1[c].T.astype(bf)),
            "x2t": np.ascontiguousarray(x2c.T.astype(bf)),
            "g": g_b, "wv": wv_b, "wo": wo_b,
            "mb": np.where(kidx < len(idx), np.float32(0.0),
                           np.float32(-1.0e9)),
        })
    return nc, in_maps


def kernel(x1, x2, maskSeq, wq, wk, wv, wo, **_unused):
    from concourse.bass_utils import run_bass_kernel_spmd

    nc, in_maps = _prepare(x1, x2, maskSeq, wq, wk, wv, wo)
    res = run_bass_kernel_spmd(nc, in_maps, core_ids=list(range(N_CORES)))
    return np.stack([res.results[c]["out"] for c in range(N_CORES)], axis=0)


# revision 19
# speedup vs baseline: 1.2243x; 1.0283x over previous
"""Bass/Tile Trainium2 kernel for nn_Attention_7284264534326.

Single-head attention, B=8, S=2048, D=1024:
    q = (x1 @ wq) * D**-0.5 ; k = x2 @ wk ; v = x2 @ wv
    a = softmax(q @ k^T + mask * -1e9, axis=-1)
    out = relu(a @ v) @ wo

Sharding: data-parallel over batch; one batch element per NeuronCore (8 cores).

Structural optimizations vs a direct implementation:
  - maskSeq is per-KEY and constant across queries, and exp(x - 1e9) == 0
    exactly in f32.  Masked keys contribute nothing, so the host compacts x2
    to its unmasked rows (padded to K_CAP, a multiple of 128), shrinking the
    k/v projections, score matmul, and a@v matmul from S=2048 keys to K_CAP
    (~1152 for ~50% masking).  Pad keys get an additive -1e9 bias so their
    exp is exactly 0.
  - Associativity fold: scores = (x1 wq s)(x2c wk)^T = x1 G x2c^T with
    G = s * wq wk^T computed on the HOST.  The device computes
    T1 = x2c @ G^T (over the ~1152 compacted keys only) and then
    scores^T = T1^T-contraction against x1^T directly — the entire
    q-projection GEMM disappears from the device.
  - x1 and compacted-x2 are transposed to [D, *] layout on the host, so the
    kernel needs no PE transposes: every matmul operand arrives in
    [contraction-on-partitions, free] layout.
  - Per-core dataflow (all matmul operands bf16, PSUM accumulation f32):
      T1T[d',k] = matmul(lhsT=G[d,d'], rhs=x2cT[d,k]);  V[k,e] likewise
      from wv.  Per 512-query tile:
      scores^T[k,q] = matmul(lhsT=T1T[:,d',k-blk], rhs=x1T[d',q]); exp fused
      into ACT evacuation with the additive mask as a per-partition bias.
      softmax denominator: DVE sums the exp chunk tiles in f32, then one
      tiny f32 matmul per 128-query block against a ones-vector reduces over
      partitions (keeps the heavy reduction off the PE).
      yU^T[e,q] = matmul(lhsT=V, rhs=exp^T); relu on ACT evacuation
      (normalization deferred: relu(y/d) == relu(y)/d for d>0).
      out[q,f] = matmul(lhsT=relu^T, rhs=wo) scaled by 1/denom on evacuation.
"""

import numpy as np
from contextlib import ExitStack

B, S, D = 8, 2048, 1024
P = 128
DC = D // P       # 8 chunks of the depth/contraction dim
EC = D // P       # 8 chunks of the embedding dim
Q_TILE = 512      # queries per tile (max moving free dim)
NQT = S // Q_TILE # 4
N_CORES = 8
QSCALE = float(D) ** -0.5  # folded into G on the host

_nc_cache = {}


def _build(k_cap):
    import concourse.tile as tile
    from concourse import bacc, mybir

    f32 = mybir.dt.float32
    bf16 = mybir.dt.bfloat16
    AF = mybir.ActivationFunctionType
    KC = k_cap // P

    nc = bacc.Bacc("TRN2", target_bir_lowering=False, debug=False,
                   enable_asserts=False, num_devices=N_CORES)

    x1t = nc.dram_tensor("x1t", [D, S], bf16, kind="ExternalInput").ap()
    x2t = nc.dram_tensor("x2t", [D, k_cap], bf16, kind="ExternalInput").ap()
    g = nc.dram_tensor("g", [D, D], bf16, kind="ExternalInput").ap()
    wv = nc.dram_tensor("wv", [D, D], bf16, kind="ExternalInput").ap()
    wo = nc.dram_tensor("wo", [D, D], bf16, kind="ExternalInput").ap()
    mb = nc.dram_tensor("mb", [P, KC], f32, kind="ExternalInput").ap()
    out = nc.dram_tensor("out", [S, D], f32, kind="ExternalOutput").ap()

    with tile.TileContext(nc) as tc, ExitStack() as ctx:
        persist = ctx.enter_context(tc.tile_pool(name="persist", bufs=1))

        t1 = persist.tile([P, DC, k_cap], bf16, name="t1")   # [d', k] by d'-chunk
        V = persist.tile([P, KC, D], bf16, name="V")         # [k, e] by k-chunk
        wo_sb = persist.tile([P, DC, D], bf16, name="wo_sb")
        maskbias = persist.tile([P, KC], f32, name="maskbias")
        ones_f = persist.tile([P, 1], f32, name="ones_f")

        x1pool = ctx.enter_context(tc.tile_pool(name="x1pool", bufs=2))
        ppsum = ctx.enter_context(tc.tile_pool(name="ppsum", bufs=2, space="PSUM"))

        nc.vector.memset(ones_f, 1.0)

        x1tiles = {}

        def load_x1(qt, eng=None):
            t = x1pool.tile([P, DC, Q_TILE], bf16, name="x1s", tag="x1s")
            (eng or nc.sync).dma_start(
                t, x1t[:, qt * Q_TILE:(qt + 1) * Q_TILE].rearrange(
                    "(c p) s -> p c s", p=P))
            x1tiles[qt] = t

        # ============ phase 1: T1 and V (x2-path weights scoped) ============
        with ExitStack() as p1:
            kvpool = p1.enter_context(tc.tile_pool(name="kvpool", bufs=1))
            x2sb = kvpool.tile([P, DC, k_cap], bf16, name="x2sb")
            g_sb = kvpool.tile([P, DC, D], bf16, name="g_sb")
            wv_sb = kvpool.tile([P, DC, D], bf16, name="wv_sb")

            # T1 key-slices: near-equal widths <= 512.  Keeping every slice
            # >= ~256 wide matters: a matmul's LDWEIGHTS (128 cy) only hides
            # under a stream of >= its length, so short slices run at half
            # rate.  (1152 -> 3x384.)
            nsl = -(-k_cap // 512)
            base, rem = divmod(k_cap, nsl)
            kslices = []
            k0 = 0
            for i in range(nsl):
                w = base + (1 if i < rem else 0)
                kslices.append((k0, w))
                k0 += w

            # Per-queue DMA sustains only ~140-160 GB/s (gpsimd's software
            # DGE is the slowest), so schedule the loads on explicit queues
            # in priority order.  The first T1 chains (ec 0..3, slice0) need
            # only G cols 0:512 (split in eighths over the two fast HW
            # queues) + the small x2 slice0 (gpsimd).
            # Each queue admits only ~5 outstanding DMAs, every piece pays
            # ~0.8us queue overhead, and the first transfer ~6.5us warmup,
            # so: few medium pieces, each queue's critical share sized to
            # complete at the same moment (~13.5us), stragglers avoided.
            SY, SC, GP = nc.sync, nc.scalar, nc.gpsimd

            def x2_half(k0, w, h, eng):
                eng.dma_start(
                    x2sb[:, 4 * h:4 * h + 4, k0:k0 + w],
                    x2t[4 * h * P:(4 * h + 4) * P, k0:k0 + w]
                    .rearrange("(c p) k -> p c k", p=P))

            def x2_quarter(k0, w, dh, eng):
                eng.dma_start(
                    x2sb[:, 2 * dh:2 * dh + 2, k0:k0 + w],
                    x2t[2 * dh * P:(2 * dh + 2) * P, k0:k0 + w]
                    .rearrange("(c p) k -> p c k", p=P))

            def g_half(e0, h, eng):
                eng.dma_start(
                    g_sb[:, 4 * h:4 * h + 4, e0:e0 + 512],
                    g[4 * h * P:(4 * h + 4) * P, e0:e0 + 512]
                    .rearrange("(c p) e -> p c e", p=P))

            ks = kslices + [None] * (3 - len(kslices))
            # critical: G cols 0:512 + x2 slice0, landing together
            g_half(0, 0, SY)
            g_half(0, 1, SC)
            if ks[0]:
                k0, w = ks[0]
                x2_quarter(k0, w, 2, GP)
                x2_quarter(k0, w, 3, GP)
                x2_quarter(k0, w, 0, SY)
                x2_quarter(k0, w, 1, SC)
            # then the next key-slice, G's other half, the rest
            if ks[1]:
                k0, w = ks[1]
                x2_half(k0, w, 0, SY)
                x2_half(k0, w, 1, SC)
            g_half(512, 0, SY)
            g_half(512, 1, SC)
            for sl in kslices[2:]:
                k0, w = sl
                x2_half(k0, w, 0, GP)
                x2_half(k0, w, 1, GP)
            for h, eng in enumerate((GP, GP)):
                eng.dma_start(
                    wv_sb[:, 4 * h:4 * h + 4, :],
                    wv[4 * h * P:(4 * h + 4) * P, :]
                    .rearrange("(c p) e -> p c e", p=P))
            load_x1(0)  # sync: overlaps phase 1 compute
            GP.dma_start(maskbias, mb)
            for h, eng in enumerate((SY, SC)):
                eng.dma_start(
                    wo_sb[:, 4 * h:4 * h + 4, :],
                    wo[4 * h * P:(4 * h + 4) * P, :]
                    .rearrange("(c p) e -> p c e", p=P))

            # eh0-half chains first: they only need G columns 0:512, so the
            # PE can run while G's other half and wv still stream in.
            for eh in range(2):
                for k0, w in kslices:
                    for ec in range(4 * eh, 4 * eh + 4):
                        pq = ppsum.tile([P, 512], f32, name="pq", tag="pp")
                        for dc in range(DC):
                            nc.tensor.matmul(
                                pq[:, :w],
                                lhsT=g_sb[:, dc, ec * P:(ec + 1) * P],
                                rhs=x2sb[:, dc, k0:k0 + w],
                                start=(dc == 0), stop=(dc == DC - 1))
                        nc.scalar.activation(
                            out=t1[:, ec, k0:k0 + w], in_=pq[:, :w],
                            func=AF.Copy)

            for kc in range(KC):
                for eh in range(2):
                    pv = ppsum.tile([P, 512], f32, name="pq", tag="pp")
                    for dc in range(DC):
                        nc.tensor.matmul(
                            pv, lhsT=x2sb[:, dc, kc * P:(kc + 1) * P],
                            rhs=wv_sb[:, dc, eh * 512:(eh + 1) * 512],
                            start=(dc == 0), stop=(dc == DC - 1))
                    # DVE evacuation keeps the ACT engine free for T1/exp
                    nc.vector.tensor_copy(
                        out=V[:, kc, eh * 512:(eh + 1) * 512], in_=pv)

        # ============ phase 2: per-512-query-tile attention ============
        epool = ctx.enter_context(tc.tile_pool(name="epool", bufs=2))
        zpool = ctx.enter_context(tc.tile_pool(name="zpool", bufs=2))
        opool = ctx.enter_context(tc.tile_pool(name="opool", bufs=3))
        dpool = ctx.enter_context(tc.tile_pool(name="dpool", bufs=2))
        rpool = ctx.enter_context(tc.tile_pool(name="rpool", bufs=2))
        spsum = ctx.enter_context(tc.tile_pool(name="spsum", bufs=2, space="PSUM"))
        ypsum = ctx.enter_context(tc.tile_pool(name="ypsum", bufs=2, space="PSUM"))
        opsum = ctx.enter_context(tc.tile_pool(name="opsum", bufs=2, space="PSUM"))

        for qt in range(NQT):
            q0 = qt * Q_TILE
            if qt + 1 < NQT:
                load_x1(qt + 1)
            x1s = x1tiles.pop(qt)

            # scores^T + fused exp(mask-biased): contraction over d' with
            # T1 stationary and the raw x1^T slice moving
            expt = epool.tile([P, KC, Q_TILE], bf16, name="expt", tag="expt")
            for kc in range(KC):
                ps = spsum.tile([P, Q_TILE], f32, name="ps", tag="ps")
                for dc in range(DC):
                    nc.tensor.matmul(
                        ps, lhsT=t1[:, dc, kc * P:(kc + 1) * P],
                        rhs=x1s[:, dc, :],
                        start=(dc == 0), stop=(dc == DC - 1))
                nc.scalar.activation(
                    out=expt[:, kc, :], in_=ps, func=AF.Exp,
                    bias=maskbias[:, kc:kc + 1], scale=1.0)

            # partial softmax denominators on DVE (f32), [k-part, q]
            dsum = dpool.tile([P, Q_TILE], f32, name="dsum", tag="dsum")
            if KC == 1:
                nc.vector.tensor_copy(out=dsum, in_=expt[:, 0, :])
            else:
                nc.vector.tensor_add(dsum, expt[:, 0, :], expt[:, 1, :])
                for kc in range(2, KC):
                    nc.vector.tensor_add(dsum, dsum, expt[:, kc, :])

            # yU^T = V^T @ exp^T, relu on evacuation
            zt = zpool.tile([P, EC, Q_TILE], bf16, name="zt", tag="zt")
            for ec in range(EC):
                py = ypsum.tile([P, Q_TILE], f32, name="py", tag="py")
                for kc in range(KC):
                    nc.tensor.matmul(
                        py, lhsT=V[:, kc, ec * P:(ec + 1) * P],
                        rhs=expt[:, kc, :],
                        start=(kc == 0), stop=(kc == KC - 1))
                nc.scalar.activation(out=zt[:, ec, :], in_=py, func=AF.Relu)

            # finish the denominators: reduce dsum over partitions per
            # 128-query block (f32 matmul against ones; borrows an opsum slot)
            pd = opsum.tile([P, 512], f32, name="po", tag="po")
            for qs in range(Q_TILE // P):
                nc.tensor.matmul(
                    pd[:, qs:qs + 1], lhsT=dsum[:, qs * P:(qs + 1) * P],
                    rhs=ones_f, start=True, stop=True)
            recip = rpool.tile([P, Q_TILE // P], f32, name="recip", tag="recip")
            nc.vector.reciprocal(recip, pd[:, 0:Q_TILE // P])

            # output projection, normalized on evacuation
            for qs in range(Q_TILE // P):
                osb = opool.tile([P, D], f32, name="osb", tag="osb")
                for fh in range(2):
                    po = opsum.tile([P, 512], f32, name="po", tag="po")
                    for ec in range(EC):
                        nc.tensor.matmul(
                            po, lhsT=zt[:, ec, qs * P:(qs + 1) * P],
                            rhs=wo_sb[:, ec, fh * 512:(fh + 1) * 512],
                            start=(ec == 0), stop=(ec == EC - 1))
                    nc.scalar.activation(
                        out=osb[:, fh * 512:(fh + 1) * 512], in_=po,
                        func=AF.Copy, scale=recip[:, qs:qs + 1])
                    # per-half store: the DMA overlaps the other half's evac
                    nc.sync.dma_start(
                        out[q0 + qs * P: q0 + (qs + 1) * P,
                            fh * 512:(fh + 1) * 512],
                        osb[:, fh * 512:(fh + 1) * 512])

    nc.compile()
    return nc


def _prepare(x1, x2, maskSeq, wq, wk, wv, wo):
    """Host-side prep: compact keys, fold wq@wk^T, transpose activations."""
    import ml_dtypes
    bf = ml_dtypes.bfloat16

    x1 = np.asarray(x1, dtype=np.float32)
    x2 = np.asarray(x2, dtype=np.float32)
    msk = np.asarray(maskSeq, dtype=np.int32).reshape(B, S)
    counts = [int(np.count_nonzero(msk[c] == 0)) for c in range(B)]
    k_cap = max(P, -(-max(counts) // P) * P)
    KC = k_cap // P

    if k_cap not in _nc_cache:
        _nc_cache[k_cap] = _build(k_cap)
    nc = _nc_cache[k_cap]

    # device computes T1 = x2c @ G^T from g's rows as the contraction dim,
    # so send G^T = s * wk @ wq^T
    g_f = QSCALE * (np.asarray(wk, dtype=np.float32) @
                    np.asarray(wq, dtype=np.float32).T)
    g_b = np.ascontiguousarray(g_f.astype(bf))
    wv_b = np.ascontiguousarray(np.asarray(wv, dtype=np.float32).astype(bf))
    wo_b = np.ascontiguousarray(np.asarray(wo, dtype=np.float32).astype(bf))

    kidx = np.arange(k_cap).reshape(KC, P).T  # kidx[p, kc] = kc*128 + p
    in_maps = []
    for c in range(B):
        idx = np.flatnonzero(msk[c] == 0)
        x2c = np.zeros((k_cap, D), dtype=np.float32)
        x2c[:len(idx)] = x2[c][idx]
        in_maps.append({
            "x1t": np.ascontiguousarray(x1[c].T.astype(bf)),
            "x2t": np.ascontiguousarray(x2c.T.astype(bf)),
            "g": g_b, "wv": wv_b, "wo": wo_b,
            "mb": np.where(kidx < len(idx), np.float32(0.0),
                           np.float32(-1.0e9)),
        })
    return nc, in_maps


def kernel(x1, x2, maskSeq, wq, wk, wv, wo, **_unused):
    from concourse.bass_utils import run_bass_kernel_spmd

    nc, in_maps = _prepare(x1, x2, maskSeq, wq, wk, wv, wo)
    res = run_bass_kernel_spmd(nc, in_maps, core_ids=list(range(N_CORES)))
    return np.stack([res.results[c]["out"] for c in range(N_CORES)], axis=0)
